# revision 2
# baseline (speedup 1.0000x reference)
"""CRF Viterbi decode — v2: Act-producer + DVE stt/reduce hybrid forward.

Forward per step (partitions p=(ic*32+b); group ic reduces i in [64ic,64ic+64)):
  - DVE stt chain for k in [0, N_DVE) -> acc [128,256]
  - Act produces planes k in [N_DVE, 64): ch[:, :, kk] = trans_rep[:,k,:] + s[:,k]
    (strided writes into chunk tiles [128, 256, nk])
  - DVE tensor_reduce max per chunk (contiguous-k inner axis); partials merged
    via TT max (PARTIAL_PLANE=False) or by riding as plane 0 of next chunk
  - TT max with acc -> macc; 4-group combine: Act shift-copies groups 1..3 to
    partition-base 0 tiles, then 3 aligned TT maxes -> m32 [32,256]
  - fold: Act copies m32 col-blocks to partition offsets, DVE adds em -> s_next
  - lat store via DMA (off critical path)
Backtrack: recompute-argmax chains as v1, but argmax via max8/max_index
(first-index ties verified on HW) and idx bitcast uint32->int32 feeds the
SWDGE gather directly.
"""

import numpy as np

B, T, K = 256, 512, 256
NCORES = 8
BLOC = B // NCORES  # 32
OUT_T = T + 2  # 514
NCHAIN = 2

N_DVE = 16          # k's on DVE stt chain
CHUNK = 8           # planes per Act chunk (PSUM tile = CHUNK KB/partition)
CH_SPACE = "PSUM"   # chunk tile space: Act strided->PSUM measured 477ns/plane


def build_program(t_steps: int = T):
    from contextlib import ExitStack

    import concourse.bass as bass
    import concourse.tile as tile
    from concourse import bacc, mybir

    FP32 = mybir.dt.float32
    INT32 = mybir.dt.int32
    UINT32 = mybir.dt.uint32
    A = mybir.AluOpType

    n_act = 64 - N_DVE
    chunk_sizes = []
    rem = n_act
    while rem > 0:
        chunk_sizes.append(min(CHUNK, rem))
        rem -= min(CHUNK, rem)

    nc = bacc.Bacc("TRN2", target_bir_lowering=False, num_devices=NCORES)

    em_f_d = nc.dram_tensor("em_f", [t_steps, 128, 64], FP32, kind="ExternalInput")
    trans_rep_d = nc.dram_tensor("trans_rep", [64, 128, K], FP32, kind="ExternalInput")
    transT_d = nc.dram_tensor("transT", [K, K], FP32, kind="ExternalInput")
    tags_d = nc.dram_tensor("tags", [BLOC, OUT_T], INT32, kind="ExternalOutput")
    lat_d = nc.dram_tensor("lat", [t_steps, 128, 64], FP32)

    with tile.TileContext(nc) as tc:
        with ExitStack() as ctx:
            static_pool = ctx.enter_context(tc.tile_pool(name="static", bufs=1))
            state_pool = ctx.enter_context(tc.tile_pool(name="state", bufs=3))
            acc_pool = ctx.enter_context(tc.tile_pool(name="acc", bufs=2))
            m_pool = ctx.enter_context(tc.tile_pool(name="m", bufs=2))
            g_pool = ctx.enter_context(tc.tile_pool(name="g", bufs=2))
            h_pool = ctx.enter_context(tc.tile_pool(name="h", bufs=2))
            if CH_SPACE == "PSUM":
                ch_pool = ctx.enter_context(tc.psum_pool(name="ch", bufs=2))
            else:
                ch_pool = ctx.enter_context(tc.tile_pool(name="ch", bufs=2))
            em_pool = ctx.enter_context(tc.tile_pool(name="em", bufs=6))
            bt_pool = ctx.enter_context(tc.tile_pool(name="bt", bufs=12))
            sm_pool = ctx.enter_context(tc.tile_pool(name="sm", bufs=6))

            # ---- static loads ----
            trans_rep = static_pool.tile([128, 64, K], FP32)
            nc.sync.dma_start(trans_rep[:], trans_rep_d.ap().transpose([1, 0, 2]))
            CHB = [(BLOC * c // NCHAIN, BLOC * (c + 1) // NCHAIN) for c in range(NCHAIN)]
            tags_u = [
                static_pool.tile([hi - lo, OUT_T], UINT32, name=f"tagsu{c}", tag=f"tagsu{c}")
                for c, (lo, hi) in enumerate(CHB)
            ]
            for tu in tags_u:
                nc.vector.memset(tu[:], 0)

            em_tiles = {}

            def em_load(t):
                if t >= t_steps:
                    return
                em_t = em_pool.tile([128, 64], FP32)
                nc.sync.dma_start(em_t[:], em_f_d.ap()[t])
                em_tiles[t] = em_t

            # ---- t = 0 ----
            s = state_pool.tile([128, 64], FP32)
            nc.sync.dma_start(s[:], em_f_d.ap()[0])
            nc.sync.dma_start(lat_d.ap()[0], em_f_d.ap()[0])
            for t in (1, 2, 3):
                em_load(t)

            # ---- forward scan ----
            for t in range(1, t_steps):
                # DVE stt head
                acc = acc_pool.tile([128, K], FP32)
                nc.vector.tensor_scalar(
                    acc[:], trans_rep[:, 0, :], s[:, 0:1], None, op0=A.add
                )
                em_load(t + 3)
                for k in range(1, N_DVE):
                    nc.vector.scalar_tensor_tensor(
                        acc[:], trans_rep[:, k, :], s[:, k : k + 1], acc[:],
                        op0=A.add, op1=A.max,
                    )
                # Act producer planes (all issued up-front on Act queue)
                k0 = N_DVE
                chs = []
                for nk in chunk_sizes:
                    ch = ch_pool.tile([128, K, nk], FP32)
                    for kk in range(nk):
                        nc.scalar.add(
                            ch[:, :, kk],
                            trans_rep[:, k0 + kk, :],
                            s[:, k0 + kk : k0 + kk + 1],
                        )
                    chs.append(ch)
                    k0 += nk
                # DVE chunk reduces, merged into running max (starts from stt acc)
                macc = acc
                for ci, ch in enumerate(chs):
                    mc = m_pool.tile([128, K], FP32, name=f"mc{ci}", tag="mc")
                    nc.vector.tensor_reduce(
                        mc[:], ch[:], axis=mybir.AxisListType.X, op=A.max)
                    m2 = m_pool.tile([128, K], FP32, name=f"mm{ci}", tag="mm")
                    nc.vector.tensor_tensor(out=m2[:], in0=macc[:], in1=mc[:], op=A.max)
                    macc = m2
                # 4-group combine via Act shift-copies + aligned TT maxes
                g1 = g_pool.tile([32, K], FP32, name="g1", tag="g1")
                g2 = g_pool.tile([32, K], FP32, name="g2", tag="g2")
                g3 = g_pool.tile([32, K], FP32, name="g3", tag="g3")
                nc.scalar.copy(g1[:], macc[32:64, :])
                nc.scalar.copy(g2[:], macc[64:96, :])
                nc.scalar.copy(g3[:], macc[96:128, :])
                c1 = g_pool.tile([32, K], FP32, name="c1", tag="c1")
                nc.vector.tensor_tensor(out=c1[:], in0=macc[0:32, :], in1=g1[:], op=A.max)
                nc.vector.tensor_tensor(out=c1[:], in0=c1[:], in1=g2[:], op=A.max)
                m32 = g_pool.tile([32, K], FP32, name="m32", tag="m32")
                nc.vector.tensor_tensor(out=m32[:], in0=c1[:], in1=g3[:], op=A.max)
                # fold: Act copies col-blocks to partition offsets, DVE adds em
                em_t = em_tiles.pop(t)
                h = h_pool.tile([128, 64], FP32)
                s = state_pool.tile([128, 64], FP32)
                for ic in range(4):
                    nc.scalar.copy(
                        h[ic * 32 : (ic + 1) * 32, :], m32[:, ic * 64 : (ic + 1) * 64]
                    )
                    nc.vector.tensor_tensor(
                        out=s[ic * 32 : (ic + 1) * 32, :],
                        in0=h[ic * 32 : (ic + 1) * 32, :],
                        in1=em_t[ic * 32 : (ic + 1) * 32, :],
                        op=A.add,
                    )
                nc.sync.dma_start(lat_d.ap()[t], s[:])

            # ---- backtrack ----
            def lat_rows(t, lo, hi):
                return lat_d.ap()[t].rearrange("(ic bb) k -> bb ic k", ic=4)[lo:hi]

            def argmax_step(val, t_col, c):
                nb = CHB[c][1] - CHB[c][0]
                m8 = sm_pool.tile([nb, 8], FP32, name=f"m8{c}", tag=f"m8{c}")
                nc.vector.max(m8[:], val[:])
                i8 = sm_pool.tile([nb, 8], UINT32, name=f"i8{c}", tag=f"i8{c}")
                nc.vector.max_index(i8[:], m8[:], val[:])
                nc.vector.tensor_copy(tags_u[c][:, t_col : t_col + 1], i8[:, 0:1])
                return i8

            idxs = [None] * NCHAIN
            for c, (lo, hi) in enumerate(CHB):
                sv = bt_pool.tile([hi - lo, K], FP32, name=f"sv{c}", tag=f"sv{c}")
                nc.sync.dma_start(sv[:], lat_rows(t_steps - 1, lo, hi))
                idxs[c] = argmax_step(sv, t_steps - 1, c)

            for t in range(t_steps - 2, -1, -1):
                svs = []
                for c, (lo, hi) in enumerate(CHB):
                    sv = bt_pool.tile([hi - lo, K], FP32, name=f"svl{c}", tag=f"sv{c}")
                    eng = nc.sync if c % 2 == 0 else nc.scalar
                    eng.dma_start(sv[:], lat_rows(t, lo, hi))
                    nc.gpsimd.indirect_dma_start(
                        out=sv[:],
                        out_offset=None,
                        in_=transT_d.ap(),
                        in_offset=bass.IndirectOffsetOnAxis(
                            ap=idxs[c][:, 0:1].bitcast(INT32), axis=0
                        ),
                        compute_op=A.add,
                    )
                    svs.append(sv)
                for c in range(NCHAIN):
                    idxs[c] = argmax_step(svs[c], t, c)

            # ---- output ----
            for c, (lo, hi) in enumerate(CHB):
                nc.sync.dma_start(tags_d.ap()[lo:hi, :], tags_u[c][:].bitcast(INT32))

    nc.compile()
    return nc


def _prep_inputs(emissions, transitions, t_steps: int = T):
    emissions = np.ascontiguousarray(emissions[:, :t_steps, :], dtype=np.float32)
    transitions = np.ascontiguousarray(transitions, dtype=np.float32)

    tr = transitions.reshape(4, 64, K).transpose(1, 0, 2)
    trans_rep = np.broadcast_to(tr[:, :, None, :], (64, 4, BLOC, K)).reshape(64, 128, K)
    trans_rep = np.ascontiguousarray(trans_rep)
    transT = np.ascontiguousarray(transitions.T)

    in_maps = []
    for c in range(NCORES):
        em_c = emissions[c * BLOC : (c + 1) * BLOC]
        em_f = np.ascontiguousarray(
            em_c.reshape(BLOC, t_steps, 4, 64)
            .transpose(1, 2, 0, 3)
            .reshape(t_steps, 128, 64)
        )
        in_maps.append({"em_f": em_f, "trans_rep": trans_rep, "transT": transT})
    return in_maps


def kernel(emissions, transitions, mask, max_sequence_length):
    from concourse.bass_utils import run_bass_kernel_spmd

    emissions = np.asarray(emissions)
    transitions = np.asarray(transitions)
    mask = np.asarray(mask)

    nc = build_program(T)
    in_maps = _prep_inputs(emissions, transitions, T)
    res = run_bass_kernel_spmd(nc, in_maps, list(range(NCORES)))
    tags = np.concatenate([res.results[c]["tags"] for c in range(NCORES)], axis=0)
    tags = tags.astype(np.int32)
    tags[:, :T] *= mask.astype(np.int32)
    return tags
